# revision 1
# baseline (speedup 1.0000x reference)
"""Trainium2 Bass kernel for nn_DiTEmb_dynamics (DiT embedding with pairwise
Fourier positional encoding), distributed data-parallel over batch across 8
NeuronCores.

Math (per batch b, N=256 nodes):
  x = center(xh[..., :3]);  h = xh[..., 3:9]
  xh_emb = [x, h] @ W_xh + b_xh                                   [N, 64]
  d_ij = ||x_i - x_j||;  feat = [sin(2 pi f_k d), cos(2 pi f_k d)] (k<64)
  pe_i = (sum_j m_j feat_ij) @ (W_pos / Nc) + (256/Nc) b_pos      [N, 192]
  out = concat([xh_emb, pe]) * m_i                                [N, 256]

Key restructurings vs the reference:
  - sum_j is moved BEFORE the @W_pos matmul (linearity), eliminating the
    [N, N, 192] intermediate and its 25.8 GFLOP matmul.
  - sin/cos are evaluated on the Scalar(ACT) engine in 16 instructions of
    [128, 4096] via 8-frequency partition grouping: distance rows are
    replicated 8x across partition groups (PE selector matmuls) and a single
    custom DVE op computes w = frac(f_k d + 1/8) per octet; ACT then applies
    sin(2 pi w -/+ pi/4) which equals sin/cos(2 pi f_k d) exactly (the 1/8
    turn shift keeps both branches' spline arguments within +-1.25 pi).
  - the j-reduction is mask-weighted PE selector-matmuls (fp16 operands,
    fp32 PSUM accumulation) at full PE rate.
  - distances are translation-invariant, so they are computed from raw
    coordinates; the masked centering (needed only for xh_emb) runs off the
    critical path during the main loop.
"""

import sys

sys.path.insert(0, "/opt/trn_rl_repo")

import numpy as np

import concourse.bass as bass
import concourse.bacc as bacc
import concourse.tile as tile
import concourse.dve_ops as dve_ops
from concourse import mybir, bass_utils
from concourse.dve_spec import Spec, Src0, Src1, C0, C1, C2, lower, sq
from concourse.dve_uop import DveOpSpec

# ---------------------------------------------------------------- constants
SIGMA = 100.0
M = 64
N_DIMS = 3
XH_IN = 9
XH_HID = 64
POS_HID = 192
B = 8
NN = 256            # nodes per graph
P = 128             # SBUF partitions
O = 8               # frequency octets (8 freqs each)
KG = 8              # freq groups per ACT instruction
JS = 16             # j-rows per partition group
JT = 16             # j-tiles
TWO_PI = float(2.0 * np.pi)
MAGIC = 12582912.0  # 1.5 * 2^23: (y + MAGIC) - MAGIC == round(y) for |y| < 2^22
SHIFT = 0.125       # shared 1/8-turn shift; sin bias -pi/4, cos bias +pi/4
F32 = mybir.dt.float32
F16 = mybir.dt.float16
AF = mybir.ActivationFunctionType
ALU = mybir.AluOpType

_FREQS = (SIGMA ** (np.arange(M, dtype=np.float32) / M)).astype(np.float32)

# packed-constants layouts (see _host_pack / _build_program)
PK1_W = 160          # [128, .]: xhp 0:18, mcol 18:20, scal 20:24, freqs 24:32, eye 32:160
PK2_H, PK2_W = 16, 960   # [16, .]: wxh r0:9 c0:64, maskrow r0 c64:320, sel3 r0:3 c320:704, bx r0 c704:768, bp r0 c768:960
PK3_W = 1408         # [128, .]: dsel 0:1024, wposa 1024:1216, wposb 1216:1408


# ------------------------------------------------------- custom DVE ops
def _register(name, spec_body, reference, rd1):
    for op in dve_ops.OPS:
        if op.name == name:
            return op
    shas = {}
    for ver in ("v3", "v4"):
        s = DveOpSpec(
            name=name,
            opcode=dve_ops._CUSTOM_DVE_ROW_BASE + len(dve_ops.OPS),
            uops=lower(Spec(body=spec_body, reference=reference), ver=ver),
            rd1_en=rd1,
        )
        shas[ver] = s.sha(ver)
    op = dve_ops.DveOp(
        name, Spec(body=spec_body, reference=reference), subdim=False, uops_sha=shas
    )
    dve_ops.OPS.append(op)
    dve_ops.CUSTOM_DVE_SPECS[name] = op.spec
    dve_ops._SUB_OPCODE_FOR_NAME[name] = (
        dve_ops._CUSTOM_DVE_ROW_BASE + len(dve_ops.OPS) - 1
    )
    return op


def _frac_ref(in0, in1, s0, s1, imm2):
    y = (in0 * s0 + s1).astype(np.float32)
    r = ((y + np.float32(imm2)).astype(np.float32) - np.float32(imm2)).astype(
        np.float32
    )
    return (y - r).astype(np.float32)


_y = Src0 * C0 + C1
FRAC_OP = _register("FRAC_AFFINE_ANT", _y - ((_y + C2) - C2), _frac_ref, rd1=False)
SQDIFF_OP = _register(
    "SQDIFF_ANT",
    sq(Src0 - C0),
    lambda in0, in1, s0, s1, imm2: ((in0 - s0).astype(np.float32) ** 2).astype(
        np.float32
    ),
    rd1=False,
)
SQDIFF_ACC_OP = _register(
    "SQDIFF_ACC_ANT",
    sq(Src0 - C0) + Src1,
    lambda in0, in1, s0, s1, imm2: (
        ((in0 - s0).astype(np.float32) ** 2).astype(np.float32) + in1
    ).astype(np.float32),
    rd1=True,
)
SQDIFF_ACC_EPS_OP = _register(
    "SQDIFF_ACC_EPS_ANT",
    (sq(Src0 - C0) + Src1) + C1,
    lambda in0, in1, s0, s1, imm2: (
        (((in0 - s0).astype(np.float32) ** 2).astype(np.float32) + in1) + s1
    ).astype(np.float32),
    rd1=True,
)
TTMS_OP = _register(
    "TT_MULT_SCALE_ANT",
    (Src0 * Src1) * C0,
    lambda in0, in1, s0, s1, imm2: ((in0 * in1).astype(np.float32) * s0).astype(
        np.float32
    ),
    rd1=True,
)


# ------------------------------------------------------------ program build
def _build_program(n_octets=O, repeats=1, do_sin=True, do_reduce=True):
    nc = bacc.Bacc("TRN2", target_bir_lowering=False, debug=False, num_devices=B)

    t_pk1 = nc.dram_tensor("pk1", [P, PK1_W], F32, kind="ExternalInput")
    t_pk2 = nc.dram_tensor("pk2", [PK2_H, PK2_W], F32, kind="ExternalInput")
    t_pk3 = nc.dram_tensor("pk3", [P, PK3_W], F32, kind="ExternalInput")
    t_selm = nc.dram_tensor("selm32", [P, JT, 4, 32], F16, kind="ExternalInput")
    t_out = nc.dram_tensor("out_b", [NN, NN], F32, kind="ExternalOutput")

    with tile.TileContext(nc) as tc:
        with (
            tc.tile_pool(name="consts", bufs=1) as cp,
            tc.tile_pool(name="work", bufs=2) as wp,
            tc.tile_pool(name="wfrac", bufs=3) as fp,
            tc.tile_pool(name="souts", bufs=4) as sop,
            tc.tile_pool(name="ps", bufs=2, space="PSUM") as pp,
            tc.tile_pool(name="psD", bufs=2, space="PSUM") as ppD,
            tc.tile_pool(name="psF", bufs=1, space="PSUM") as ppF,
        ):
            # ---- load packed constants (2 queues, critical pack first) ---
            pk1 = cp.tile([P, PK1_W], F32, tag="pk1")
            nc.sync.dma_start(out=pk1, in_=t_pk1.ap())
            pk2 = cp.tile([PK2_H, PK2_W], F32, tag="pk2")
            nc.gpsimd.dma_start(out=pk2, in_=t_pk2.ap())
            pk3 = cp.tile([P, PK3_W], F32, tag="pk3")
            nc.gpsimd.dma_start(out=pk3, in_=t_pk3.ap())
            c_selm = cp.tile([P, JT, 4, 32], F16, tag="selm")
            nc.gpsimd.dma_start(out=c_selm, in_=t_selm.ap())

            c_xhp = pk1[:, 0:18].rearrange("p (a b) -> p a b", a=2)
            c_mcol = pk1[:, 18:20]
            c_scal = pk1[:, 20:24]
            c_freqs = pk1[:, 24:32]
            c_eye = pk1[:, 32:160]
            c_wxh = pk2[0:XH_IN, 0:64]
            c_maskrow = pk2[0:1, 64:320]
            c_sel3 = pk2[0:N_DIMS, 320:704].rearrange("p (a b) -> p a b", a=3)
            c_bx = pk2[0:1, 704:768]
            c_bp = pk2[0:1, 768:960]
            c_dsel = pk3[:, 0:1024].rearrange("p (a b) -> p a b", a=8)
            c_wpos_a = pk3[0:M, 1024:1216]
            c_wpos_b = pk3[0:M, 1216:1408]

            c_ones1 = cp.tile([1, P], F32, tag="ones1")
            nc.vector.memset(c_ones1, 1.0)
            c_onescol = cp.tile([P, 1], F32, tag="onescol")
            nc.vector.memset(c_onescol, 1.0)
            # warm the PE clock (HAM) before the first real matmuls
            ps_warm = pp.tile([P, P], F32, tag="psmisc")
            for _wi in range(4):
                nc.tensor.matmul(
                    ps_warm, lhsT=c_ones1, rhs=c_ones1, start=True, stop=True
                )

            import contextlib

            if repeats > 1:
                loop_cm = tc.For_i(
                    0, repeats, 1,
                    hint_engines=(
                        mybir.EngineType.Activation, mybir.EngineType.DVE,
                        mybir.EngineType.PE, mybir.EngineType.SP,
                        mybir.EngineType.Pool,
                    ),
                )
            else:
                loop_cm = contextlib.nullcontext()
            with loop_cm:
                # ---- XHT = xh^T via PE transpose --------------------------
                # Distances are translation-invariant: use RAW coords here;
                # the masked centering only matters for xh_emb (computed
                # later, off the critical path).
                xht = cp.tile([XH_IN, NN], F32, tag="xht")
                for ic in range(2):
                    ps_t = pp.tile([XH_IN, P], F32, tag="psmisc")
                    nc.tensor.matmul(
                        ps_t, lhsT=c_xhp[:, ic, :], rhs=c_eye, is_transpose=True,
                        start=True, stop=True,
                    )
                    nc.vector.tensor_copy(out=xht[:, ic * P:(ic + 1) * P], in_=ps_t)

                # ---- pairwise distances -----------------------------------
                # xib[c] = broadcast of raw-x row c over 128 partitions
                xib = ppD.tile([P, N_DIMS, NN], F32, tag="psdr4")
                for c in range(N_DIMS):
                    nc.tensor.matmul(
                        xib[:, c, :], lhsT=c_sel3[:, c, :], rhs=xht[0:N_DIMS, :],
                        start=True, stop=True,
                    )

                sqrt_insts = []
                dsb = cp.tile([P, 2, NN], F32, tag="dsb")  # [j-part, jc, i]
                for jc in range(2):
                    # xcp = transpose of raw-x chunk -> [128, 3]
                    xcp = pp.tile([P, N_DIMS], F32, tag="psmisc")
                    nc.tensor.matmul(
                        xcp, lhsT=xht[0:N_DIMS, jc * P:(jc + 1) * P],
                        rhs=c_eye[0:N_DIMS, 0:N_DIMS], is_transpose=True,
                        start=True, stop=True,
                    )
                    # xeps = sum_c (x_i[c] - x_j[c])^2 + 1e-12 (ref order)
                    d2 = wp.tile([P, NN], F32, tag="d2")
                    nc.vector._custom_dve(
                        SQDIFF_OP, out=d2, in0=xib[:, 0, :], s0=xcp[:, 0:1]
                    )
                    d2b = wp.tile([P, NN], F32, tag="d2b")
                    nc.vector._custom_dve(
                        SQDIFF_ACC_OP, out=d2b, in0=xib[:, 1, :], in1=d2,
                        s0=xcp[:, 1:2],
                    )
                    xeps = wp.tile([P, NN], F32, tag="xeps")
                    nc.vector._custom_dve(
                        SQDIFF_ACC_EPS_OP, out=xeps, in0=xib[:, 2, :], in1=d2b,
                        s0=xcp[:, 2:3], s1=1e-12,
                    )
                    # y0 = ACT sqrt, then one fused Newton step:
                    # dsb = 0.5*y0 + (xeps * (1/y0)) * 0.5
                    y0 = wp.tile([P, NN], F32, tag="y0")
                    sqrt_insts.append(
                        nc.scalar.activation(out=y0, in_=xeps, func=AF.Sqrt)
                    )
                    r0 = wp.tile([P, NN], F32, tag="r0")
                    nc.vector.reciprocal(out=r0, in_=y0)
                    q = wp.tile([P, NN], F32, tag="q")
                    nc.vector._custom_dve(
                        TTMS_OP, out=q, in0=xeps, in1=r0, s0=0.5
                    )
                    nc.vector.affine_then_add(
                        out=dsb[:, jc, :], in0=y0, in1=q, scale=0.5, bias=0.0
                    )

                # trigger the Sin table-set load while ACT is idle (the
                # real sins otherwise queue it behind the drep copies)
                dsin = wp.tile([1, 4], F32, tag="dsin")
                i_dsin = nc.scalar.activation(
                    out=dsin, in_=c_scal[0:1, 0:4], func=AF.Sin, scale=1.0,
                    bias=0.0,
                )
                tile.add_dep_helper(
                    i_dsin.ins, sqrt_insts[-1].ins, sync=False,
                    reason="sin table preload after last sqrt",
                )

                # ---- replicate distance rows 8x across partition groups,
                # interleaved with the first/last octet's frac+sin so the ACT
                # stream starts early and drains late without gaps ----------
                drep = cp.tile([P, JT, NN], F32, tag="drep")
                ps_Fs = ppF.tile([M, NN], F32, tag="psFs")
                ps_Fc = ppF.tile([M, NN], F32, tag="psFc")

                def build_drep(j0, j1, on_act):
                    # one [128, j1-j0, 256] PSUM group -> single copy; early
                    # copies ride the otherwise-idle Scalar engine. Returns
                    # the PSUM group so octet 0's frac can read it directly
                    # (skipping the copy on the stream's critical feed path).
                    ps4 = ppD.tile([P, 4, NN], F32, tag="psdr4")
                    for m in range(j1 - j0):
                        jt = j0 + m
                        nc.tensor.matmul(
                            ps4[:, m, :], lhsT=c_dsel[:, jt % O, :],
                            rhs=dsb[:, jt // O, :], start=True, stop=True,
                        )
                    seg = ps4[:, 0:j1 - j0, :]
                    if on_act:
                        nc.scalar.copy(out=drep[:, j0:j1, :], in_=seg)
                    else:
                        nc.vector.tensor_copy(out=drep[:, j0:j1, :], in_=seg)
                    return seg

                def emit_part(o, j0, j1, src=None):
                    w = fp.tile([P, j1 - j0, NN], F32, tag=f"w{j1 - j0}")
                    frac_inst = nc.vector._custom_dve(
                        FRAC_OP, out=w,
                        in0=src if src is not None else drep[:, j0:j1, :],
                        s0=c_freqs[:, o:o + 1], s1=SHIFT, imm2=MAGIC,
                    )
                    frac_insts.append(frac_inst)
                    if not do_sin:
                        return
                    for half in range(2):  # 0: sin, 1: cos
                        ps_half = ps_Fs if half == 0 else ps_Fc
                        base = 32 * (o // 4)
                        qq = o % 4
                        sv = sop.tile([P, j1 - j0, NN], F16, tag=f"sv{j1 - j0}")
                        nc.scalar.activation(
                            out=sv, in_=w, func=AF.Sin, scale=TWO_PI,
                            bias=c_scal[:, half:half + 1],
                        )
                        if not do_reduce:
                            continue
                        for jt in range(j0, j1):
                            nc.tensor.matmul(
                                ps_half[base:base + 32, :],
                                lhsT=c_selm[:, jt, qq, :],
                                rhs=sv[:, jt - j0, :],
                                start=(qq == 0 and jt == 0),
                                stop=(qq == 3 and jt == JT - 1),
                            )

                frac_insts = []
                # o=0 in quarters, each right after its drep group lands, so
                # the ACT stream starts as early as possible
                for g in range(4):
                    seg = build_drep(4 * g, 4 * g + 4, on_act=(g <= 1))
                    emit_part(0, 4 * g, 4 * g + 4, src=seg)
                if n_octets > 2:
                    # o=1 split too: its halves interleave with o=0's tail
                    # on DVE, removing the ACT bubble at the transition
                    emit_part(1, 0, 8)
                    emit_part(1, 8, JT)
                for o in range(2, n_octets - 1):
                    emit_part(o, 0, JT)
                if n_octets > 1:
                    emit_part(n_octets - 1, 0, 8)
                    emit_part(n_octets - 1, 8, JT)

                # ---- centering for xh_emb (off critical path) --------------
                xm0 = wp.tile([P, N_DIMS], F32, tag="xm0")
                xm1 = wp.tile([P, N_DIMS], F32, tag="xm1")
                i_xm0 = nc.gpsimd.tensor_scalar(
                    out=xm0, in0=c_xhp[:, 0, 0:N_DIMS], scalar1=c_mcol[:, 0:1],
                    scalar2=None, op0=ALU.mult,
                )
                i_xm1 = nc.gpsimd.tensor_scalar(
                    out=xm1, in0=c_xhp[:, 1, 0:N_DIMS], scalar1=c_mcol[:, 1:2],
                    scalar2=None, op0=ALU.mult,
                )
                if False and len(frac_insts) > 3:
                    # keep the centering chain out of the critical distance /
                    # frac startup window (same-engine ordering, no sync)
                    gate = frac_insts[3]
                    tile.add_dep_helper(
                        i_xm0.ins, gate.ins, sync=False,
                        reason="defer centering past frac startup",
                    )
                    tile.add_dep_helper(
                        i_xm1.ins, gate.ins, sync=False,
                        reason="defer centering past frac startup",
                    )
                ps_mean = pp.tile([1, N_DIMS], F32, tag="psmisc")
                nc.tensor.matmul(
                    ps_mean, lhsT=c_onescol, rhs=xm0, start=True, stop=False
                )
                nc.tensor.matmul(
                    ps_mean, lhsT=c_onescol, rhs=xm1, start=False, stop=True
                )
                meanrow = wp.tile([1, N_DIMS], F32, tag="meanrow")
                nc.vector.tensor_scalar(
                    out=meanrow, in0=ps_mean, scalar1=1.0 / float(NN),
                    scalar2=None, op0=ALU.mult,
                )
                ps_m3 = pp.tile([N_DIMS, 1], F32, tag="psmisc")
                nc.tensor.matmul(
                    ps_m3, lhsT=meanrow, rhs=c_eye[0:1, 0:1], is_transpose=True,
                    start=True, stop=True,
                )
                mean3 = wp.tile([N_DIMS, 1], F32, tag="mean3")
                nc.vector.tensor_copy(out=mean3, in_=ps_m3)
                ps_mask3 = pp.tile([N_DIMS, NN], F32, tag="psmisc")
                nc.tensor.matmul(
                    ps_mask3, lhsT=c_ones1[:, 0:N_DIMS], rhs=c_maskrow,
                    start=True, stop=True,
                )
                xct9 = cp.tile([XH_IN, NN], F32, tag="xct9")
                nc.gpsimd.tensor_copy(out=xct9, in_=xht)
                nc.vector.tensor_scalar(
                    out=xct9[0:N_DIMS, :], in0=xht[0:N_DIMS, :],
                    scalar1=mean3[:, 0:1], scalar2=None, op0=ALU.subtract,
                )
                nc.vector.tensor_tensor(
                    out=xct9[0:N_DIMS, :], in0=xct9[0:N_DIMS, :], in1=ps_mask3,
                    op=ALU.mult,
                )

                if do_sin and do_reduce:
                    f_sa = cp.tile([M, NN], F32, tag="fsa")
                    nc.vector.tensor_copy(out=f_sa, in_=ps_Fs)
                    f_sc = cp.tile([M, NN], F32, tag="fsc")
                    nc.vector.tensor_copy(out=f_sc, in_=ps_Fc)
                else:
                    f_sa = cp.tile([M, NN], F32, tag="fsa")
                    nc.vector.memset(f_sa, 0.0)
                    f_sc = cp.tile([M, NN], F32, tag="fsc")
                    nc.vector.memset(f_sc, 0.0)

                # ---- tail: xh_emb, pe matmul, mask, store ------------------
                out_sb = wp.tile([P, 2, NN], F32, tag="outsb")
                for ic in range(2):
                    ps_emb = pp.tile([P, XH_HID], F32, tag="psmisc")
                    nc.tensor.matmul(
                        ps_emb, lhsT=xct9[:, ic * P:(ic + 1) * P],
                        rhs=c_wxh, start=True, stop=False,
                    )
                    nc.tensor.matmul(
                        ps_emb, lhsT=c_ones1, rhs=c_bx, start=False, stop=True
                    )
                    ps_pe = pp.tile([P, POS_HID], F32, tag="psmisc")
                    nc.tensor.matmul(
                        ps_pe, lhsT=f_sa[:, ic * P:(ic + 1) * P], rhs=c_wpos_a,
                        start=True, stop=False,
                    )
                    nc.tensor.matmul(
                        ps_pe, lhsT=f_sc[:, ic * P:(ic + 1) * P], rhs=c_wpos_b,
                        start=False, stop=False,
                    )
                    nc.tensor.matmul(
                        ps_pe, lhsT=c_ones1, rhs=c_bp, start=False, stop=True
                    )
                    nc.vector.tensor_scalar(
                        out=out_sb[:, ic, 0:XH_HID], in0=ps_emb,
                        scalar1=c_mcol[:, ic:ic + 1], scalar2=None, op0=ALU.mult,
                    )
                    nc.vector.tensor_scalar(
                        out=out_sb[:, ic, XH_HID:NN], in0=ps_pe,
                        scalar1=c_mcol[:, ic:ic + 1], scalar2=None, op0=ALU.mult,
                    )
                nc.sync.dma_start(
                    out=t_out.ap()[0:P, :], in_=out_sb[:, 0, :]
                )
                nc.sync.dma_start(
                    out=t_out.ap()[P:NN, :], in_=out_sb[:, 1, :]
                )

    nc.compile()
    return nc


_PROGRAM = None


def _get_program():
    global _PROGRAM
    if _PROGRAM is None:
        _PROGRAM = _build_program()
    return _PROGRAM


# ------------------------------------------------------------- host wrapper
def _host_pack(xh_b, mask, W_xh, b_xh, W_pos, b_pos):
    """Build the per-core packed input tensors."""
    n_count = np.float32(mask.sum())

    pk1 = np.zeros((P, PK1_W), np.float32)
    pk1[:, 0:18] = xh_b.reshape(2, P, XH_IN).transpose(1, 0, 2).reshape(P, 18)
    pk1[:, 18:20] = mask.reshape(2, P).T
    pk1[:, 20] = -np.pi / 4
    pk1[:, 21] = +np.pi / 4
    pk1[:, 22] = 1e-12
    po = np.arange(P)[:, None] // JS
    oo = np.arange(O)[None, :]
    pk1[:, 24:32] = _FREQS[(oo * KG + po).astype(np.int64)]
    pk1[:, 32:160] = np.eye(P, dtype=np.float32)

    pk2 = np.zeros((PK2_H, PK2_W), np.float32)
    pk2[0:XH_IN, 0:64] = W_xh
    pk2[0, 64:320] = mask
    sel3 = np.zeros((N_DIMS, N_DIMS, P), np.float32)
    for c in range(N_DIMS):
        sel3[c, c, :] = 1.0
    pk2[0:N_DIMS, 320:704] = sel3.reshape(N_DIMS, N_DIMS * P)
    pk2[0, 704:768] = b_xh
    pk2[0, 768:960] = b_pos * (NN / n_count)

    pk3 = np.zeros((P, PK3_W), np.float32)
    qs = np.arange(P)[:, None, None]
    jjs = np.arange(O)[None, :, None]
    pvec = np.arange(P)[None, None, :]
    dsel = (qs == jjs * JS + (pvec % JS)).astype(np.float32)
    pk3[:, 0:1024] = dsel.reshape(P, O * P)
    wpos = (W_pos / n_count).astype(np.float32)
    pk3[0:M, 1024:1216] = wpos[0:M]
    pk3[0:M, 1216:1408] = wpos[M:2 * M]

    ps = np.arange(P)
    selm32 = np.zeros((P, JT, 4, 32), np.float16)
    for jt in range(JT):
        vals = mask[jt * JS + (ps % JS)]
        for q in range(4):
            selm32[ps, jt, q, q * KG + ps // JS] = vals

    return {"pk1": pk1, "pk2": pk2, "pk3": pk3, "selm32": selm32}


def _make_in_maps(xh, node_mask, W_xh, b_xh, W_pos, b_pos):
    return [
        _host_pack(
            xh[b].astype(np.float32),
            node_mask[b, :, 0].astype(np.float32),
            np.asarray(W_xh, np.float32),
            np.asarray(b_xh, np.float32),
            np.asarray(W_pos, np.float32),
            np.asarray(b_pos, np.float32),
        )
        for b in range(B)
    ]


def kernel(t, xh, node_mask, edge_mask, W_xh, b_xh, W_pos, b_pos):
    xh = np.asarray(xh, dtype=np.float32)
    node_mask = np.asarray(node_mask, dtype=np.float32)

    nc = _get_program()
    in_maps = _make_in_maps(xh, node_mask, W_xh, b_xh, W_pos, b_pos)
    res = bass_utils.run_bass_kernel_spmd(nc, in_maps, core_ids=list(range(B)))
    out = np.stack([res.results[b]["out_b"] for b in range(B)], axis=0)
    return out.astype(np.float32)

